# revision 31
# baseline (speedup 1.0000x reference)
"""Trainium2 Bass kernel for ALayer: out = x * box3x3(sigmoid(conv3x3(relu(conv3x3(x,w1)),w2))).

Sharding: pure data parallel over batch (32 images -> 4 per core x 8 cores).

Per-core plan:
  - x loaded per 2-image pack as [128=(img2,c64), guarded spatial] bf16 (DMA cast),
    in 8 row-chunks; an fp8e4m3 copy x8 [128, 2, BUF_W] (both packs in the DoubleRow
    K-segment dim) is made by chunked ScalarE copy-casts (keeps DMA queues free).
  - conv1 interior: fp8 DoubleRow matmuls, K=256=(img4,c64), M=64=(pack2,img2,co16),
    9 taps accumulated in PSUM, tap-outer over groups of 4 tiles to amortize weight
    loads; full-width rows (contiguous rhs, wrap garbage at x=0/127) + bf16 per-pack
    edge-column matmuls with only the valid taps.
  - relu+cast eviction (DVE): ONE [64,512] op per tile into h64 [64=(pack,img,co), .]
    fp8; dy-replicas hrep4 [96=(dy3,img2,c16), 2seg, .] built by 6 chunked shifted
    DMAs; conv2 = 3 fp8-DR matmuls (dx), K=192, both packs at once.
  - sigmoid evictions (ACT) into group 0 of per-pack Arep [128-part, 18 live];
    8 shifted DMA replicas; box+channel-broadcast in ONE bf16 matmul: K=128
    (zero-padded), M=128=(img2,c64).
  - final multiply in-place on the x tile (DVE), stored with bf16->f32 DMA cast.

All stages emitted at 16-row chunk granularity so the Tile scheduler overlaps
DMA / PE / ACT / DVE across stage boundaries.
"""

import numpy as np
import ml_dtypes

import concourse.bacc as bacc
import concourse.mybir as mybir
from concourse.tile import TileContext
from concourse.bass_utils import run_bass_kernel_spmd

BF16 = mybir.dt.bfloat16
F32 = mybir.dt.float32
FP8 = mybir.dt.float8e4

# Problem constants (hardcoded; kernel.py must be self-contained)
B, C, H, W = 32, 64, 128, 128
N_CORES = 8
B_LOC = B // N_CORES            # 4 images per core
PACKS = B_LOC // 2              # 2 two-image packs
S = H * W                       # 16384
FG = 160                        # front zero-guard (>=129 needed)
BG = 288
BUF_W = FG + S + BG             # 16832 (16-aligned for DoubleRow AP steps)
Y4 = 4                          # output rows per psum tile
NT = H // Y4                    # 32 row tiles
GRP = 4                         # conv1 tiles per weight-reuse group
NCH = 16                        # load row-chunks (8 rows each)
NHCH = 16                       # repl_h width chunks (fixed DMA issue cost)
CROWS = H // NCH                # 8
NBLK = 4                        # 32-row blocks (edges, repl_a, stores)
BROWS = H // NBLK               # 32
XI = W - 2                      # 126 interior columns
TAPORD = (4, 0, 1, 2, 3, 5, 6, 7, 8)   # Arep group -> tap index (center first)


def _pos(y, x):
    return FG + y * W + x


def _host_weights(w1, w2):
    """Precompute SBUF weight layouts (block-diagonal over image packing)."""
    w1 = np.asarray(w1, np.float32)     # [16, 64, 3, 3]
    w2 = np.asarray(w2, np.float32)     # [1, 16, 3, 3]
    bf = ml_dtypes.bfloat16
    f8 = mybir.dt.np(FP8)

    # conv1 DoubleRow M=128: psum m = dx*64 + seg*32 + il*16 + och, dx in
    # {0,1} (w1A, full-width passes) and the dx=2 taps (w1B, M=64) written
    # into the dx=0 partition block at psum cols [0,510) reading rhs +2:
    # psum0[p] = partial_dx0[p] + partial_dx2[p+2], so
    # h[p] = relu(psum0[p-1] + psum1[p]) needs only a 2-term eviction.
    w1A = np.zeros((128, 2, 3, 128), np.float32)
    w1B = np.zeros((128, 2, 3, 64), np.float32)
    for il in range(2):
        for seg in range(2):
            for ky in range(3):
                for dx in range(2):
                    m0 = dx * 64 + seg * 32 + il * 16
                    w1A[il * 64:(il + 1) * 64, seg, ky, m0:m0 + 16] = \
                        w1[:, :, ky, dx].T
                m0 = seg * 32 + il * 16
                w1B[il * 64:(il + 1) * 64, seg, ky, m0:m0 + 16] = \
                    w1[:, :, ky, 2].T
    # conv1 edges (bf16, per pack): lhsT[p=(i2,c64), t, m=(il2,co16)]
    w1L = np.zeros((128, 9, 32), np.float32)
    for i in range(2):
        for t in range(9):
            ky, kx = divmod(t, 3)
            w1L[i * 64:(i + 1) * 64, t, i * 16:(i + 1) * 16] = w1[:, :, ky, kx].T
    # conv2 DoubleRow: lhsT[p=(g3,il2,c16), seg2, kx, m] = w2[0,c,g,kx] at
    # m = 2*seg + il (M=4: psum rows = (seg,il) = A2 rows 0..3)
    w2D = np.zeros((96, 2, 3, 32), np.float32)
    for g in range(3):
        for il in range(2):
            for seg in range(2):
                for kx in range(3):
                    w2D[g * 32 + il * 16:g * 32 + il * 16 + 16, seg, kx,
                        2 * seg + il] = w2[0, :, g, kx]
    # conv2 edges (fp8 non-DR, per seg): lhsT[p=(g3,i2,c16), kx, il2]
    w2L = np.zeros((96, 3, 2), np.float32)
    for g in range(3):
        for i in range(2):
            for kx in range(3):
                w2L[g * 32 + i * 16:g * 32 + (i + 1) * 16, kx, i] = w2[0, :, g, kx]
    # box+bcast: K=36 over A2 rows (j9, seg2, il2); per-pack lhsT selects seg:
    # onesL[p=(j,seg,il), pk, e, m=(il'2,c64)] =
    #   tap-allowed(t(j), e) * (il' == il) * (seg == pk)
    onesL = np.zeros((36, 2, 3, 128), np.float32)
    for j, t in enumerate(TAPORD):
        kx = t % 3
        for seg in range(2):
            for il in range(2):
                p = 4 * j + 2 * seg + il
                onesL[p, seg, 0, il * 64:(il + 1) * 64] = 1.0
                if kx >= 1:
                    onesL[p, seg, 1, il * 64:(il + 1) * 64] = 1.0
                if kx <= 1:
                    onesL[p, seg, 2, il * 64:(il + 1) * 64] = 1.0
    return (w1A.astype(f8), w1B.astype(f8), w1L.astype(bf), w2D.astype(f8),
            w2L.astype(f8), onesL.astype(bf))


def _view(t, off, rows, cols):
    """AP over [partitions, (rows @ W-stride, cols @ 1)] at free offset `off`."""
    return t[:, off:off + (rows - 1) * W + 128].rearrange(
        "p (y x) -> p y x", x=W)[:, :, :cols]


def _view3(t, seg, off, rows, cols):
    """Same for a [P, 2, BUF_W] tensor at segment `seg`."""
    return t[:, seg, off:off + (rows - 1) * W + 128].rearrange(
        "p (y x) -> p y x", x=W)[:, :, :cols]


def _split(a, b, n=NCH):
    step = (b - a) // n
    cuts = [a + i * step for i in range(n)] + [b]
    return [(cuts[i], cuts[i + 1]) for i in range(n)]


def _build_nc():
    nc = bacc.Bacc(None, target_bir_lowering=False, debug=False)

    x_ext = nc.declare_dram_parameter("x", [B_LOC, C, H, W], F32, isOutput=False)
    out_ext = nc.declare_dram_parameter("out", [B_LOC, C, H, W], F32, isOutput=True)
    w1A_ext = nc.declare_dram_parameter("w1A", [128, 2, 3, 128], FP8, isOutput=False)
    w1B_ext = nc.declare_dram_parameter("w1B", [128, 2, 3, 64], FP8, isOutput=False)
    w1L_ext = nc.declare_dram_parameter("w1L", [128, 9, 32], BF16, isOutput=False)
    w2D_ext = nc.declare_dram_parameter("w2D", [96, 2, 3, 32], FP8, isOutput=False)
    w2L_ext = nc.declare_dram_parameter("w2L", [96, 3, 2], FP8, isOutput=False)
    onesL_ext = nc.declare_dram_parameter("onesL", [36, 2, 3, 128], BF16, isOutput=False)

    with TileContext(nc) as tc:
        with (
            tc.tile_pool(name="wpool", bufs=1) as wpool,
            tc.tile_pool(name="xpool", bufs=2) as xpool,
            tc.tile_pool(name="hpool", bufs=1) as hpool,
            tc.tile_pool(name="bigpool", bufs=2) as bigpool,
            tc.tile_pool(name="aepool", bufs=2) as aepool,
            tc.tile_pool(name="scpool", bufs=4) as scpool,
            tc.tile_pool(name="psa", bufs=3, space="PSUM") as psa_pool,
            tc.tile_pool(name="psb", bufs=2, space="PSUM") as psb_pool,
            tc.tile_pool(name="psc", bufs=3, space="PSUM") as psc_pool,
        ):
            w1A = wpool.tile([128, 2, 3, 128], FP8)
            w1B = wpool.tile([128, 2, 3, 64], FP8)
            w1L = wpool.tile([128, 9, 32], BF16)
            w2D = wpool.tile([96, 2, 3, 32], FP8)
            w2L = wpool.tile([96, 3, 2], FP8)
            onesL = wpool.tile([36, 2, 3, 128], BF16)
            for dst, src in ((w1A, w1A_ext), (w1B, w1B_ext), (w1L, w1L_ext),
                             (w2D, w2D_ext), (w2L, w2L_ext), (onesL, onesL_ext)):
                nc.sync.dma_start(out=dst[:], in_=src[:])

            x_sb = [None] * PACKS

            x8 = bigpool.tile([128, 2, BUF_W], FP8, tag="big", name="x8")
            # attention maps: row 4j+2seg+il = tap group j (j=0 center),
            # pack seg, image il. Rows 0..3 written by sigmoid evictions;
            # rows 4..35 are shifted replicas (repl_a). Only rows 0..3 guards
            # are ever read through shifts -> tiny memsets.
            A2 = bigpool.tile([36, BUF_W], BF16, tag="big", name="A2")
            h64 = hpool.tile([64, BUF_W], FP8, name="h64")
            hrep4 = hpool.tile([96, 2, BUF_W], FP8, name="hrep4")

            def guard_memsets():
                nc.vector.memset(x8[:, :, 0:FG], 0.0)
                nc.vector.memset(x8[:, :, FG + S:BUF_W], 0.0)
                nc.vector.memset(A2[0:4, 0:FG], 0.0)
                nc.vector.memset(A2[0:4, FG + S:BUF_W], 0.0)
                for t in (h64, hrep4):
                    nc.vector.memset(t[:, ..., 0:FG], 0.0)
                    nc.vector.memset(t[:, ..., FG + S:BUF_W], 0.0)

            def alloc(p):
                x_sb[p] = xpool.tile([128, BUF_W], BF16, tag="xsb", name=f"xsb{p}")

            def load(p, c):
                # f32 -> bf16 cast DMA (SWDGE); chunks overlap by one row so
                # consumers of rows [8c, 8c+8] depend only on chunks <= c
                r1 = min((c + 1) * CROWS + 1, H)
                nc.gpsimd.dma_start(
                    out=x_sb[p][:, _pos(c * CROWS, 0):_pos(r1, 0)],
                    in_=x_ext[2 * p:2 * p + 2, :, c * CROWS:r1]
                    .rearrange("b c h w -> (b c) (h w)"),
                )

            def cast8(p, b):
                # bf16 -> fp8 cast DMA (SWDGE; only gpsimd can cast),
                # 32-row blocks (+1 overlap row like the loads)
                r1 = min((b + 1) * BROWS + 1, H)
                a, e = _pos(b * BROWS, 0), _pos(r1, 0)
                nc.gpsimd.dma_start(out=x8[:, p, a:e], in_=x_sb[p][:, a:e])

            def conv1_dr(rt):
                # 6 passes into ONE psum bank: 3 ky passes at M=128 (dx 0/1)
                # full width, then 3 ky passes at M=64 (dx=2) into the dx=0
                # partition block at cols [0,510) reading the rhs at +2.
                pa = psa_pool.tile([128, 512], F32, tag="pa", name="pa")
                for ky in range(3):
                    q = _pos(rt * Y4 + ky - 1, 0)
                    nc.tensor.matmul(
                        pa[:, :],
                        w1A[:, :, ky, :],
                        x8[:, :, q:q + 512],
                        perf_mode=mybir.MatmulPerfMode.DoubleRow,
                        start=(ky == 0), stop=False,
                    )
                for ky in range(3):
                    q = _pos(rt * Y4 + ky - 1, 2)
                    nc.tensor.matmul(
                        pa[0:64, 0:510],
                        w1B[:, :, ky, :],
                        x8[:, :, q:q + 510],
                        perf_mode=mybir.MatmulPerfMode.DoubleRow,
                        start=False, stop=(ky == 2),
                    )
                # h[p] = relu(pa0[p-1] + pa1[p]) for flat p in [1,511);
                # cols x=0/127 are wrap-garbage, overwritten by the edge path.
                # DVE can read only ONE psum input -> 3-op pipeline across
                # ACT (psum copy), DVE (add), Pool (relu + fp8 cast).
                q0 = _pos(rt * Y4, 0)
                t1 = scpool.tile([64, 510], BF16, tag="sc", name="t1")
                nc.scalar.activation(t1[:, :], pa[64:128, 1:511],
                                     mybir.ActivationFunctionType.Copy)
                nc.vector.tensor_add(t1[:, :], pa[0:64, 0:510], t1[:, :])
                nc.vector.tensor_scalar_max(
                    h64[:, q0 + 1:q0 + 511], t1[:, :], 0.0)

            def conv1_edges(p, u):
                # both edge columns for 16 rows share one psum bank
                xs = x_sb[p]
                yc = u * 16
                pe = psb_pool.tile([32, 2, 16], F32, tag="pb", name="pe")
                for half, (col, kxs) in enumerate(((0, (1, 2)), (W - 1, (0, 1)))):
                    first = True
                    for ky in range(3):
                        for kx in kxs:
                            nc.tensor.matmul(
                                pe[:, half, :].rearrange("p (y x) -> p y x", x=1),
                                w1L[:, ky * 3 + kx, :],
                                _view(xs, _pos(yc + ky - 1, col + kx - 1), 16, 1),
                                start=first, stop=(ky == 2 and kx == kxs[-1]),
                            )
                            first = False
                # ONE relu+cast for both edge columns
                dst = h64[32 * p:32 * p + 32,
                          _pos(yc, 0):_pos(yc + 15, W - 1) + 1].rearrange(
                    "p (y x) -> p y x", x=W)[:, :, 0:W:W - 1]
                nc.vector.tensor_scalar_max(
                    dst, pe[:, :, :].rearrange("p h y -> p y h"), 0.0)

            def repl_h(c):
                # Hrep4[(g,il,c), s][pos] = h_seg_s[pos + (g-1)*W]
                # 4 coarse chunks: HWDGE issue cost is fixed per DMA
                for s in range(2):
                    a0, b0 = _split(W, BUF_W, NHCH)[c]
                    nc.sync.dma_start(out=hrep4[0:32, s, a0:b0],
                                      in_=h64[32 * s:32 * s + 32, a0 - W:b0 - W])
                    a1, b1 = _split(0, BUF_W, NHCH)[c]
                    nc.sync.dma_start(out=hrep4[32:64, s, a1:b1],
                                      in_=h64[32 * s:32 * s + 32, a1:b1])
                    a2, b2 = _split(0, BUF_W - W, NHCH)[c]
                    nc.sync.dma_start(out=hrep4[64:96, s, a2:b2],
                                      in_=h64[32 * s:32 * s + 32, a2 + W:b2 + W])

            def conv2(g2):
                # dx-outer over pairs of tiles: weight loads amortize
                tiles = [2 * g2, 2 * g2 + 1]
                pzs = [psb_pool.tile([32, 512], F32, tag="pb", name="pz")
                       for _ in tiles]
                for kx in range(3):
                    for j, rt in enumerate(tiles):
                        q = _pos(rt * Y4, kx - 1)
                        nc.tensor.matmul(
                            pzs[j][:, :],
                            w2D[:, :, kx, :],
                            hrep4[:, :, q:q + 512],
                            perf_mode=mybir.MatmulPerfMode.DoubleRow,
                            start=(kx == 0), stop=(kx == 2),
                        )
                for j, rt in enumerate(tiles):
                    y0 = rt * Y4
                    # ONE sigmoid for all 4 images (edge cols garbage; fixed
                    # below)
                    nc.scalar.activation(
                        A2[0:4, _pos(y0, 0):_pos(y0, 0) + 512],
                        pzs[j][0:4, :],
                        mybir.ActivationFunctionType.Sigmoid,
                    )

            ae_cur = [None]

            def conv2_edges(u):
                # all 4 (pack, half) edge strips share one psum tile at
                # partition base 0: [2=(il), pk2, half2, y16]
                yc = u * 16
                pz = psb_pool.tile([2, 2, 2, 16], F32, tag="pb", name="pze")
                for p in range(PACKS):
                    for half, (col, kxs) in enumerate(((0, (1, 2)),
                                                       (W - 1, (0, 1)))):
                        for j, kx in enumerate(kxs):
                            nc.tensor.matmul(
                                pz[:, p, half, :]
                                .rearrange("p (y x) -> p y x", x=1),
                                w2L[:, kx, :],
                                _view3(hrep4, p, _pos(yc, col + kx - 1), 16, 1),
                                start=(j == 0), stop=(j == len(kxs) - 1),
                            )
                # ONE sigmoid for all 4 strips into scratch, then 2 DMAs
                # scatter into A2 rows (which need base-2 access -> DMA only)
                ae = aepool.tile([2, 2, 2, 16], BF16, name="ae")
                nc.scalar.activation(
                    ae[:, :, :, :], pz[:, :, :, :],
                    mybir.ActivationFunctionType.Sigmoid,
                )
                for p in range(PACKS):
                    for half, col in enumerate((0, W - 1)):
                        dst = _view(A2, _pos(yc, col), 16, 1)[2 * p:2 * p + 2]
                        nc.scalar.dma_start(
                            out=dst,
                            in_=ae[:, p, half, :].rearrange(
                                "i (y x) -> i y x", x=1))

            def repl_a(b):
                # tap j for all 4 images at once: [4, L] shifted self-copies
                for j in range(1, 9):
                    t = TAPORD[j]
                    o = (t // 3 - 1) * W + (t % 3 - 1)
                    if o > 0:
                        a, e = _split(0, BUF_W - o, NBLK)[b]
                    else:
                        a, e = _split(-o, BUF_W, NBLK)[b]
                    nc.gpsimd.dma_start(out=A2[4 * j:4 * j + 4, a:e],
                                        in_=A2[0:4, a + o:e + o])

            def box_mul(p, rt, reuse_w=False):
                xs = x_sb[p]
                y0 = rt * Y4
                pb = psc_pool.tile([128, 504], F32, tag="pc", name="pb")
                bi = nc.tensor.matmul(
                    pb[:, :].rearrange("p (y x) -> p y x", y=Y4),
                    onesL[:, p, 0, :],
                    _view(A2, _pos(y0, 1), Y4, XI)[0:36],
                    start=True, stop=True,
                )
                if reuse_w:
                    bi.ins.ldweights = False
                v = _view(xs, _pos(y0, 1), Y4, XI)
                nc.vector.tensor_mul(
                    v, v, pb[:, :].rearrange("p (y x) -> p y x", y=Y4))

            def box_mul_edges(p, u):
                xs = x_sb[p]
                yc = u * 16
                pb = psc_pool.tile([128, 2, 16], F32, tag="pc", name="pbe")
                for half, (e, col) in enumerate(((1, 0), (2, W - 1))):
                    nc.tensor.matmul(
                        pb[:, half, :].rearrange("p (y x) -> p y x", x=1),
                        onesL[:, p, e, :],
                        _view(A2, _pos(yc, col), 16, 1)[0:36],
                        start=True, stop=True,
                    )
                # ONE multiply for both edge columns
                v = xs[:, _pos(yc, 0):_pos(yc + 15, W - 1) + 1].rearrange(
                    "p (y x) -> p y x", x=W)[:, :, 0:W:W - 1]
                nc.vector.tensor_mul(
                    v, v, pb[:, :, :].rearrange("p h y -> p y h"))

            def store(p, b):
                # bf16 -> f32 cast DMA (SWDGE), rows [b*32, (b+1)*32)
                nc.gpsimd.dma_start(
                    out=out_ext[2 * p:2 * p + 2, :, b * BROWS:(b + 1) * BROWS]
                    .rearrange("b c h w -> (b c) (h w)"),
                    in_=x_sb[p][:, _pos(b * BROWS, 0):_pos((b + 1) * BROWS, 0)],
                )

            # ---- emission: software pipeline over 16-row ticks ----
            # tick u: conv1(u) | repl_h(u-1) | conv2(u-2) | repl_a(u-4, 32-row)
            #         | box+mul(u-6) | store(u-7, 32-row)
            alloc(0)
            alloc(1)
            for p in range(PACKS):
                nc.vector.memset(x_sb[p][:, 0:FG], 0.0)
                nc.vector.memset(x_sb[p][:, FG + S:BUF_W], 0.0)
            guard_memsets()
            for b in range(NBLK):
                for c in range(4 * b, 4 * b + 4):
                    load(0, c)
                    load(1, c)
                cast8(0, b)
                cast8(1, b)

            NTK = 8   # 16-row ticks
            for u in range(NTK + 8):
                # emission order IS program order for the scheduler's
                # read-after-write semantics: keep producer chain order
                # conv1 -> repl_h -> conv2 -> repl_a -> box -> store.
                if u < NTK:
                    for rt in range(4 * u, 4 * u + 4):
                        conv1_dr(rt)
                    conv1_edges(0, u)
                    conv1_edges(1, u)
                v = u - 1
                if 0 <= v < NTK:
                    repl_h(2 * v)
                    repl_h(2 * v + 1)
                v = u - 2
                if 0 <= v < NTK:
                    conv2(2 * v)
                    conv2(2 * v + 1)
                    conv2_edges(v)
                v = u - 4
                if 0 <= v < NTK and v % 2 == 0:
                    repl_a(v // 2)
                v = u - 6
                if 0 <= v < NTK:
                    for p in range(PACKS):
                        for i, rt in enumerate(range(4 * v, 4 * v + 4)):
                            box_mul(p, rt, reuse_w=(i > 0))
                        box_mul_edges(p, v)
                v = u - 7
                if 0 <= v < NTK and v % 2 == 1:
                    store(0, v // 2)
                    store(1, v // 2)

    nc.compile()
    return nc


_CACHE = {}


def _get_nc():
    if "nc" not in _CACHE:
        _CACHE["nc"] = _build_nc()
    return _CACHE["nc"]


def _reset_device():
    """Best-effort axon terminal reset (recovers NRT_EXEC_UNIT_UNRECOVERABLE)."""
    try:
        import ctypes

        lib = ctypes.CDLL("/opt/axon/libaxon_pjrt.so")
        lib.axon_reset.restype = ctypes.c_int64
        lib.axon_reset()
    except Exception:
        pass


def _run(x, w1, w2, trace=False):
    x = np.ascontiguousarray(np.asarray(x, np.float32))
    w1A, w1B, w1L, w2D, w2L, onesL = _host_weights(w1, w2)
    nc = _get_nc()
    in_maps = []
    for k in range(N_CORES):
        in_maps.append({
            "x": x[k * B_LOC:(k + 1) * B_LOC],
            "w1A": w1A, "w1B": w1B, "w1L": w1L, "w2D": w2D, "w2L": w2L,
            "onesL": onesL,
        })
    try:
        res = run_bass_kernel_spmd(nc, in_maps, core_ids=list(range(N_CORES)),
                                   trace=trace)
    except Exception as e:
        if "unrecoverable" not in str(e).lower():
            raise
        _reset_device()
        res = run_bass_kernel_spmd(nc, in_maps, core_ids=list(range(N_CORES)),
                                   trace=trace)
    out = np.concatenate([r["out"] for r in res.results], axis=0)
    return out.astype(np.float32), res


def kernel(x, weights, w1, w2):
    out, _ = _run(x, w1, w2, trace=False)
    return out


def kernel_timed(x, weights, w1, w2):
    out, res = _run(x, w1, w2, trace=True)
    return out, res.exec_time_ns



# revision 32
# speedup vs baseline: 1.0281x; 1.0281x over previous
"""Trainium2 Bass kernel for ALayer: out = x * box3x3(sigmoid(conv3x3(relu(conv3x3(x,w1)),w2))).

Sharding: pure data parallel over batch (32 images -> 4 per core x 8 cores).

Per-core plan:
  - x loaded per 2-image pack as [128=(img2,c64), guarded spatial] bf16 (DMA cast),
    in 8 row-chunks; an fp8e4m3 copy x8 [128, 2, BUF_W] (both packs in the DoubleRow
    K-segment dim) is made by chunked ScalarE copy-casts (keeps DMA queues free).
  - conv1 interior: fp8 DoubleRow matmuls, K=256=(img4,c64), M=64=(pack2,img2,co16),
    9 taps accumulated in PSUM, tap-outer over groups of 4 tiles to amortize weight
    loads; full-width rows (contiguous rhs, wrap garbage at x=0/127) + bf16 per-pack
    edge-column matmuls with only the valid taps.
  - relu+cast eviction (DVE): ONE [64,512] op per tile into h64 [64=(pack,img,co), .]
    fp8; dy-replicas hrep4 [96=(dy3,img2,c16), 2seg, .] built by 6 chunked shifted
    DMAs; conv2 = 3 fp8-DR matmuls (dx), K=192, both packs at once.
  - sigmoid evictions (ACT) into group 0 of per-pack Arep [128-part, 18 live];
    8 shifted DMA replicas; box+channel-broadcast in ONE bf16 matmul: K=128
    (zero-padded), M=128=(img2,c64).
  - final multiply in-place on the x tile (DVE), stored with bf16->f32 DMA cast.

All stages emitted at 16-row chunk granularity so the Tile scheduler overlaps
DMA / PE / ACT / DVE across stage boundaries.
"""

import numpy as np
import ml_dtypes

import concourse.bacc as bacc
import concourse.mybir as mybir
from concourse.tile import TileContext
from concourse.bass_utils import run_bass_kernel_spmd

BF16 = mybir.dt.bfloat16
F32 = mybir.dt.float32
FP8 = mybir.dt.float8e4

# Problem constants (hardcoded; kernel.py must be self-contained)
B, C, H, W = 32, 64, 128, 128
N_CORES = 8
B_LOC = B // N_CORES            # 4 images per core
PACKS = B_LOC // 2              # 2 two-image packs
S = H * W                       # 16384
FG = 160                        # front zero-guard (>=129 needed)
BG = 288
BUF_W = FG + S + BG             # 16832 (16-aligned for DoubleRow AP steps)
Y4 = 4                          # output rows per psum tile
NT = H // Y4                    # 32 row tiles
GRP = 4                         # conv1 tiles per weight-reuse group
NCH = 16                        # load row-chunks (8 rows each)
NHCH = 16                       # repl_h width chunks (fixed DMA issue cost)
CROWS = H // NCH                # 8
NBLK = 4                        # 32-row blocks (edges, repl_a, stores)
BROWS = H // NBLK               # 32
XI = W - 2                      # 126 interior columns
TAPORD = (4, 0, 1, 2, 3, 5, 6, 7, 8)   # Arep group -> tap index (center first)


def _pos(y, x):
    return FG + y * W + x


def _host_weights(w1, w2):
    """Precompute SBUF weight layouts (block-diagonal over image packing)."""
    w1 = np.asarray(w1, np.float32)     # [16, 64, 3, 3]
    w2 = np.asarray(w2, np.float32)     # [1, 16, 3, 3]
    bf = ml_dtypes.bfloat16
    f8 = mybir.dt.np(FP8)

    # conv1 DoubleRow M=128: psum m = dx*64 + seg*32 + il*16 + och, dx in
    # {0,1} (w1A, full-width passes) and the dx=2 taps (w1B, M=64) written
    # into the dx=0 partition block at psum cols [0,510) reading rhs +2:
    # psum0[p] = partial_dx0[p] + partial_dx2[p+2], so
    # h[p] = relu(psum0[p-1] + psum1[p]) needs only a 2-term eviction.
    w1A = np.zeros((128, 2, 3, 128), np.float32)
    w1B = np.zeros((128, 2, 3, 64), np.float32)
    for il in range(2):
        for seg in range(2):
            for ky in range(3):
                for dx in range(2):
                    m0 = dx * 64 + seg * 32 + il * 16
                    w1A[il * 64:(il + 1) * 64, seg, ky, m0:m0 + 16] = \
                        w1[:, :, ky, dx].T
                m0 = seg * 32 + il * 16
                w1B[il * 64:(il + 1) * 64, seg, ky, m0:m0 + 16] = \
                    w1[:, :, ky, 2].T
    # conv1 edges (bf16, per pack): lhsT[p=(i2,c64), t, m=(il2,co16)]
    w1L = np.zeros((128, 9, 32), np.float32)
    for i in range(2):
        for t in range(9):
            ky, kx = divmod(t, 3)
            w1L[i * 64:(i + 1) * 64, t, i * 16:(i + 1) * 16] = w1[:, :, ky, kx].T
    # conv2 DoubleRow: lhsT[p=(g3,il2,c16), seg2, kx, m] = w2[0,c,g,kx] at
    # m = 2*seg + il (M=4: psum rows = (seg,il) = A2 rows 0..3)
    w2D = np.zeros((96, 2, 3, 32), np.float32)
    for g in range(3):
        for il in range(2):
            for seg in range(2):
                for kx in range(3):
                    w2D[g * 32 + il * 16:g * 32 + il * 16 + 16, seg, kx,
                        2 * seg + il] = w2[0, :, g, kx]
    # conv2 edges (fp8 non-DR, per seg): lhsT[p=(g3,i2,c16), kx, il2]
    w2L = np.zeros((96, 3, 2), np.float32)
    for g in range(3):
        for i in range(2):
            for kx in range(3):
                w2L[g * 32 + i * 16:g * 32 + (i + 1) * 16, kx, i] = w2[0, :, g, kx]
    # box+bcast: K=36 over A2 rows (j9, seg2, il2); per-pack lhsT selects seg:
    # onesL[p=(j,seg,il), pk, e, m=(il'2,c64)] =
    #   tap-allowed(t(j), e) * (il' == il) * (seg == pk)
    onesL = np.zeros((36, 2, 3, 128), np.float32)
    for j, t in enumerate(TAPORD):
        kx = t % 3
        for seg in range(2):
            for il in range(2):
                p = 4 * j + 2 * seg + il
                onesL[p, seg, 0, il * 64:(il + 1) * 64] = 1.0
                if kx >= 1:
                    onesL[p, seg, 1, il * 64:(il + 1) * 64] = 1.0
                if kx <= 1:
                    onesL[p, seg, 2, il * 64:(il + 1) * 64] = 1.0
    return (w1A.astype(f8), w1B.astype(f8), w1L.astype(bf), w2D.astype(f8),
            w2L.astype(f8), onesL.astype(bf))


def _view(t, off, rows, cols):
    """AP over [partitions, (rows @ W-stride, cols @ 1)] at free offset `off`."""
    return t[:, off:off + (rows - 1) * W + 128].rearrange(
        "p (y x) -> p y x", x=W)[:, :, :cols]


def _view3(t, seg, off, rows, cols):
    """Same for a [P, 2, BUF_W] tensor at segment `seg`."""
    return t[:, seg, off:off + (rows - 1) * W + 128].rearrange(
        "p (y x) -> p y x", x=W)[:, :, :cols]


def _split(a, b, n=NCH):
    step = (b - a) // n
    cuts = [a + i * step for i in range(n)] + [b]
    return [(cuts[i], cuts[i + 1]) for i in range(n)]


def _build_nc():
    nc = bacc.Bacc(None, target_bir_lowering=False, debug=False)

    x_ext = nc.declare_dram_parameter("x", [B_LOC, C, H, W], F32, isOutput=False)
    out_ext = nc.declare_dram_parameter("out", [B_LOC, C, H, W], F32, isOutput=True)
    w1A_ext = nc.declare_dram_parameter("w1A", [128, 2, 3, 128], FP8, isOutput=False)
    w1B_ext = nc.declare_dram_parameter("w1B", [128, 2, 3, 64], FP8, isOutput=False)
    w1L_ext = nc.declare_dram_parameter("w1L", [128, 9, 32], BF16, isOutput=False)
    w2D_ext = nc.declare_dram_parameter("w2D", [96, 2, 3, 32], FP8, isOutput=False)
    w2L_ext = nc.declare_dram_parameter("w2L", [96, 3, 2], FP8, isOutput=False)
    onesL_ext = nc.declare_dram_parameter("onesL", [36, 2, 3, 128], BF16, isOutput=False)

    with TileContext(nc) as tc:
        with (
            tc.tile_pool(name="wpool", bufs=1) as wpool,
            tc.tile_pool(name="xpool", bufs=2) as xpool,
            tc.tile_pool(name="hpool", bufs=1) as hpool,
            tc.tile_pool(name="bigpool", bufs=2) as bigpool,
            tc.tile_pool(name="aepool", bufs=2) as aepool,
            tc.tile_pool(name="scpool", bufs=4) as scpool,
            tc.tile_pool(name="psa", bufs=3, space="PSUM") as psa_pool,
            tc.tile_pool(name="psb", bufs=2, space="PSUM") as psb_pool,
            tc.tile_pool(name="psc", bufs=3, space="PSUM") as psc_pool,
        ):
            w1A = wpool.tile([128, 2, 3, 128], FP8)
            w1B = wpool.tile([128, 2, 3, 64], FP8)
            w1L = wpool.tile([128, 9, 32], BF16)
            w2D = wpool.tile([96, 2, 3, 32], FP8)
            w2L = wpool.tile([96, 3, 2], FP8)
            onesL = wpool.tile([36, 2, 3, 128], BF16)
            for dst, src in ((w1A, w1A_ext), (w1B, w1B_ext), (w1L, w1L_ext),
                             (w2D, w2D_ext), (w2L, w2L_ext), (onesL, onesL_ext)):
                nc.sync.dma_start(out=dst[:], in_=src[:])

            x_sb = [None] * PACKS

            x8 = bigpool.tile([128, 2, BUF_W], FP8, tag="big", name="x8")
            # attention maps: row 4j+2seg+il = tap group j (j=0 center),
            # pack seg, image il. Rows 0..3 written by sigmoid evictions;
            # rows 4..35 are shifted replicas (repl_a). Only rows 0..3 guards
            # are ever read through shifts -> tiny memsets.
            A2 = bigpool.tile([36, BUF_W], BF16, tag="big", name="A2")
            h64 = hpool.tile([64, BUF_W], FP8, name="h64")
            hrep4 = hpool.tile([96, 2, BUF_W], FP8, name="hrep4")

            def guard_memsets():
                nc.vector.memset(x8[:, :, 0:FG], 0.0)
                nc.vector.memset(x8[:, :, FG + S:BUF_W], 0.0)
                nc.vector.memset(A2[0:4, 0:FG], 0.0)
                nc.vector.memset(A2[0:4, FG + S:BUF_W], 0.0)
                for t in (h64, hrep4):
                    nc.vector.memset(t[:, ..., 0:FG], 0.0)
                    nc.vector.memset(t[:, ..., FG + S:BUF_W], 0.0)

            def alloc(p):
                x_sb[p] = xpool.tile([128, BUF_W], BF16, tag="xsb", name=f"xsb{p}")

            def load(p, c):
                # f32 -> bf16 cast DMA (SWDGE); chunks overlap by one row so
                # consumers of rows [8c, 8c+8] depend only on chunks <= c
                r1 = min((c + 1) * CROWS + 1, H)
                nc.gpsimd.dma_start(
                    out=x_sb[p][:, _pos(c * CROWS, 0):_pos(r1, 0)],
                    in_=x_ext[2 * p:2 * p + 2, :, c * CROWS:r1]
                    .rearrange("b c h w -> (b c) (h w)"),
                )

            def cast8(p, c):
                # bf16 -> fp8 copy-cast, split across ScalarE/VectorE
                r1 = min((c + 1) * CROWS + 1, H)
                a, b = _pos(c * CROWS, 0), _pos(r1, 0)
                if (2 * p + c) % 2 == 0:
                    nc.scalar.activation(x8[:, p, a:b], x_sb[p][:, a:b],
                                         mybir.ActivationFunctionType.Copy)
                else:
                    nc.vector.tensor_copy(x8[:, p, a:b], x_sb[p][:, a:b])

            def conv1_dr(rt):
                # 6 passes into ONE psum bank: 3 ky passes at M=128 (dx 0/1)
                # full width, then 3 ky passes at M=64 (dx=2) into the dx=0
                # partition block at cols [0,510) reading the rhs at +2.
                pa = psa_pool.tile([128, 512], F32, tag="pa", name="pa")
                for ky in range(3):
                    q = _pos(rt * Y4 + ky - 1, 0)
                    nc.tensor.matmul(
                        pa[:, :],
                        w1A[:, :, ky, :],
                        x8[:, :, q:q + 512],
                        perf_mode=mybir.MatmulPerfMode.DoubleRow,
                        start=(ky == 0), stop=False,
                    )
                for ky in range(3):
                    q = _pos(rt * Y4 + ky - 1, 2)
                    nc.tensor.matmul(
                        pa[0:64, 0:510],
                        w1B[:, :, ky, :],
                        x8[:, :, q:q + 510],
                        perf_mode=mybir.MatmulPerfMode.DoubleRow,
                        start=False, stop=(ky == 2),
                    )
                # h[p] = relu(pa0[p-1] + pa1[p]) for flat p in [1,511);
                # cols x=0/127 are wrap-garbage, overwritten by the edge path.
                # DVE can read only ONE psum input -> 3-op pipeline across
                # ACT (psum copy), DVE (add), Pool (relu + fp8 cast).
                q0 = _pos(rt * Y4, 0)
                t1 = scpool.tile([64, 510], BF16, tag="sc", name="t1")
                nc.scalar.activation(t1[:, :], pa[64:128, 1:511],
                                     mybir.ActivationFunctionType.Copy)
                nc.vector.tensor_add(t1[:, :], pa[0:64, 0:510], t1[:, :])
                nc.vector.tensor_scalar_max(
                    h64[:, q0 + 1:q0 + 511], t1[:, :], 0.0)

            def conv1_edges(p, u):
                # both edge columns for 16 rows share one psum bank
                xs = x_sb[p]
                yc = u * 16
                pe = psb_pool.tile([32, 2, 16], F32, tag="pb", name="pe")
                for half, (col, kxs) in enumerate(((0, (1, 2)), (W - 1, (0, 1)))):
                    first = True
                    for ky in range(3):
                        for kx in kxs:
                            nc.tensor.matmul(
                                pe[:, half, :].rearrange("p (y x) -> p y x", x=1),
                                w1L[:, ky * 3 + kx, :],
                                _view(xs, _pos(yc + ky - 1, col + kx - 1), 16, 1),
                                start=first, stop=(ky == 2 and kx == kxs[-1]),
                            )
                            first = False
                # ONE relu+cast for both edge columns
                dst = h64[32 * p:32 * p + 32,
                          _pos(yc, 0):_pos(yc + 15, W - 1) + 1].rearrange(
                    "p (y x) -> p y x", x=W)[:, :, 0:W:W - 1]
                nc.vector.tensor_scalar_max(
                    dst, pe[:, :, :].rearrange("p h y -> p y h"), 0.0)

            def repl_h(c):
                # Hrep4[(g,il,c), s][pos] = h_seg_s[pos + (g-1)*W]
                # 4 coarse chunks: HWDGE issue cost is fixed per DMA
                for s in range(2):
                    a0, b0 = _split(W, BUF_W, NHCH)[c]
                    nc.sync.dma_start(out=hrep4[0:32, s, a0:b0],
                                      in_=h64[32 * s:32 * s + 32, a0 - W:b0 - W])
                    a1, b1 = _split(0, BUF_W, NHCH)[c]
                    nc.sync.dma_start(out=hrep4[32:64, s, a1:b1],
                                      in_=h64[32 * s:32 * s + 32, a1:b1])
                    a2, b2 = _split(0, BUF_W - W, NHCH)[c]
                    nc.sync.dma_start(out=hrep4[64:96, s, a2:b2],
                                      in_=h64[32 * s:32 * s + 32, a2 + W:b2 + W])

            def conv2(g2):
                # dx-outer over pairs of tiles: weight loads amortize
                tiles = [2 * g2, 2 * g2 + 1]
                pzs = [psb_pool.tile([32, 512], F32, tag="pb", name="pz")
                       for _ in tiles]
                for kx in range(3):
                    for j, rt in enumerate(tiles):
                        q = _pos(rt * Y4, kx - 1)
                        nc.tensor.matmul(
                            pzs[j][:, :],
                            w2D[:, :, kx, :],
                            hrep4[:, :, q:q + 512],
                            perf_mode=mybir.MatmulPerfMode.DoubleRow,
                            start=(kx == 0), stop=(kx == 2),
                        )
                for j, rt in enumerate(tiles):
                    y0 = rt * Y4
                    # ONE sigmoid for all 4 images (edge cols garbage; fixed
                    # below)
                    nc.scalar.activation(
                        A2[0:4, _pos(y0, 0):_pos(y0, 0) + 512],
                        pzs[j][0:4, :],
                        mybir.ActivationFunctionType.Sigmoid,
                    )

            ae_cur = [None]

            def conv2_edges(u):
                # all 4 (pack, half) edge strips share one psum tile at
                # partition base 0: [2=(il), pk2, half2, y16]
                yc = u * 16
                pz = psb_pool.tile([2, 2, 2, 16], F32, tag="pb", name="pze")
                for p in range(PACKS):
                    for half, (col, kxs) in enumerate(((0, (1, 2)),
                                                       (W - 1, (0, 1)))):
                        for j, kx in enumerate(kxs):
                            nc.tensor.matmul(
                                pz[:, p, half, :]
                                .rearrange("p (y x) -> p y x", x=1),
                                w2L[:, kx, :],
                                _view3(hrep4, p, _pos(yc, col + kx - 1), 16, 1),
                                start=(j == 0), stop=(j == len(kxs) - 1),
                            )
                # ONE sigmoid for all 4 strips into scratch, then 2 DMAs
                # scatter into A2 rows (which need base-2 access -> DMA only)
                ae = aepool.tile([2, 2, 2, 16], BF16, name="ae")
                nc.scalar.activation(
                    ae[:, :, :, :], pz[:, :, :, :],
                    mybir.ActivationFunctionType.Sigmoid,
                )
                for p in range(PACKS):
                    for half, col in enumerate((0, W - 1)):
                        dst = _view(A2, _pos(yc, col), 16, 1)[2 * p:2 * p + 2]
                        nc.scalar.dma_start(
                            out=dst,
                            in_=ae[:, p, half, :].rearrange(
                                "i (y x) -> i y x", x=1))

            def repl_a(b):
                # tap j for all 4 images at once: [4, L] shifted self-copies
                for j in range(1, 9):
                    t = TAPORD[j]
                    o = (t // 3 - 1) * W + (t % 3 - 1)
                    if o > 0:
                        a, e = _split(0, BUF_W - o, NBLK)[b]
                    else:
                        a, e = _split(-o, BUF_W, NBLK)[b]
                    nc.gpsimd.dma_start(out=A2[4 * j:4 * j + 4, a:e],
                                        in_=A2[0:4, a + o:e + o])

            def box_mul(p, rt, reuse_w=False):
                xs = x_sb[p]
                y0 = rt * Y4
                pb = psc_pool.tile([128, 504], F32, tag="pc", name="pb")
                bi = nc.tensor.matmul(
                    pb[:, :].rearrange("p (y x) -> p y x", y=Y4),
                    onesL[:, p, 0, :],
                    _view(A2, _pos(y0, 1), Y4, XI)[0:36],
                    start=True, stop=True,
                )
                if reuse_w:
                    bi.ins.ldweights = False
                v = _view(xs, _pos(y0, 1), Y4, XI)
                nc.vector.tensor_mul(
                    v, v, pb[:, :].rearrange("p (y x) -> p y x", y=Y4))

            def box_mul_edges(p, u):
                xs = x_sb[p]
                yc = u * 16
                pb = psc_pool.tile([128, 2, 16], F32, tag="pc", name="pbe")
                for half, (e, col) in enumerate(((1, 0), (2, W - 1))):
                    nc.tensor.matmul(
                        pb[:, half, :].rearrange("p (y x) -> p y x", x=1),
                        onesL[:, p, e, :],
                        _view(A2, _pos(yc, col), 16, 1)[0:36],
                        start=True, stop=True,
                    )
                # ONE multiply for both edge columns
                v = xs[:, _pos(yc, 0):_pos(yc + 15, W - 1) + 1].rearrange(
                    "p (y x) -> p y x", x=W)[:, :, 0:W:W - 1]
                nc.vector.tensor_mul(
                    v, v, pb[:, :, :].rearrange("p h y -> p y h"))

            def store(p, b):
                # bf16 -> f32 cast DMA (SWDGE), rows [b*32, (b+1)*32)
                nc.gpsimd.dma_start(
                    out=out_ext[2 * p:2 * p + 2, :, b * BROWS:(b + 1) * BROWS]
                    .rearrange("b c h w -> (b c) (h w)"),
                    in_=x_sb[p][:, _pos(b * BROWS, 0):_pos((b + 1) * BROWS, 0)],
                )

            # ---- emission: software pipeline over 16-row ticks ----
            # tick u: conv1(u) | repl_h(u-1) | conv2(u-2) | repl_a(u-4, 32-row)
            #         | box+mul(u-6) | store(u-7, 32-row)
            alloc(0)
            alloc(1)
            for p in range(PACKS):
                nc.vector.memset(x_sb[p][:, 0:FG], 0.0)
                nc.vector.memset(x_sb[p][:, FG + S:BUF_W], 0.0)
            guard_memsets()
            for c in range(NCH):
                load(0, c)
                load(1, c)
                cast8(0, c)
                cast8(1, c)

            NTK = 8   # 16-row ticks
            for u in range(NTK + 8):
                # emission order IS program order for the scheduler's
                # read-after-write semantics: keep producer chain order
                # conv1 -> repl_h -> conv2 -> repl_a -> box -> store.
                if u < NTK:
                    for rt in range(4 * u, 4 * u + 4):
                        conv1_dr(rt)
                    conv1_edges(0, u)
                    conv1_edges(1, u)
                v = u - 1
                if 0 <= v < NTK:
                    repl_h(2 * v)
                    repl_h(2 * v + 1)
                v = u - 2
                if 0 <= v < NTK:
                    conv2(2 * v)
                    conv2(2 * v + 1)
                    conv2_edges(v)
                v = u - 4
                if 0 <= v < NTK and v % 2 == 0:
                    repl_a(v // 2)
                v = u - 6
                if 0 <= v < NTK:
                    for p in range(PACKS):
                        for i, rt in enumerate(range(4 * v, 4 * v + 4)):
                            box_mul(p, rt, reuse_w=(i > 0))
                        box_mul_edges(p, v)
                v = u - 7
                if 0 <= v < NTK and v % 2 == 1:
                    store(0, v // 2)
                    store(1, v // 2)

    nc.compile()
    return nc


_CACHE = {}


def _get_nc():
    if "nc" not in _CACHE:
        _CACHE["nc"] = _build_nc()
    return _CACHE["nc"]


def _reset_device():
    """Best-effort axon terminal reset (recovers NRT_EXEC_UNIT_UNRECOVERABLE)."""
    try:
        import ctypes

        lib = ctypes.CDLL("/opt/axon/libaxon_pjrt.so")
        lib.axon_reset.restype = ctypes.c_int64
        lib.axon_reset()
    except Exception:
        pass


def _run(x, w1, w2, trace=False):
    x = np.ascontiguousarray(np.asarray(x, np.float32))
    w1A, w1B, w1L, w2D, w2L, onesL = _host_weights(w1, w2)
    nc = _get_nc()
    in_maps = []
    for k in range(N_CORES):
        in_maps.append({
            "x": x[k * B_LOC:(k + 1) * B_LOC],
            "w1A": w1A, "w1B": w1B, "w1L": w1L, "w2D": w2D, "w2L": w2L,
            "onesL": onesL,
        })
    try:
        res = run_bass_kernel_spmd(nc, in_maps, core_ids=list(range(N_CORES)),
                                   trace=trace)
    except Exception as e:
        if "unrecoverable" not in str(e).lower():
            raise
        _reset_device()
        res = run_bass_kernel_spmd(nc, in_maps, core_ids=list(range(N_CORES)),
                                   trace=trace)
    out = np.concatenate([r["out"] for r in res.results], axis=0)
    return out.astype(np.float32), res


def kernel(x, weights, w1, w2):
    out, _ = _run(x, w1, w2, trace=False)
    return out


def kernel_timed(x, weights, w1, w2):
    out, res = _run(x, w1, w2, trace=True)
    return out, res.exec_time_ns



# revision 33
# speedup vs baseline: 1.0423x; 1.0138x over previous
"""Trainium2 Bass kernel for ALayer: out = x * box3x3(sigmoid(conv3x3(relu(conv3x3(x,w1)),w2))).

Sharding: pure data parallel over batch (32 images -> 4 per core x 8 cores).

Per-core plan:
  - x loaded per 2-image pack as [128=(img2,c64), guarded spatial] bf16 (DMA cast),
    in 8 row-chunks; an fp8e4m3 copy x8 [128, 2, BUF_W] (both packs in the DoubleRow
    K-segment dim) is made by chunked ScalarE copy-casts (keeps DMA queues free).
  - conv1 interior: fp8 DoubleRow matmuls, K=256=(img4,c64), M=64=(pack2,img2,co16),
    9 taps accumulated in PSUM, tap-outer over groups of 4 tiles to amortize weight
    loads; full-width rows (contiguous rhs, wrap garbage at x=0/127) + bf16 per-pack
    edge-column matmuls with only the valid taps.
  - relu+cast eviction (DVE): ONE [64,512] op per tile into h64 [64=(pack,img,co), .]
    fp8; dy-replicas hrep4 [96=(dy3,img2,c16), 2seg, .] built by 6 chunked shifted
    DMAs; conv2 = 3 fp8-DR matmuls (dx), K=192, both packs at once.
  - sigmoid evictions (ACT) into group 0 of per-pack Arep [128-part, 18 live];
    8 shifted DMA replicas; box+channel-broadcast in ONE bf16 matmul: K=128
    (zero-padded), M=128=(img2,c64).
  - final multiply in-place on the x tile (DVE), stored with bf16->f32 DMA cast.

All stages emitted at 16-row chunk granularity so the Tile scheduler overlaps
DMA / PE / ACT / DVE across stage boundaries.
"""

import numpy as np
import ml_dtypes

import concourse.bacc as bacc
import concourse.mybir as mybir
from concourse.tile import TileContext
from concourse.bass_utils import run_bass_kernel_spmd

BF16 = mybir.dt.bfloat16
F32 = mybir.dt.float32
FP8 = mybir.dt.float8e4

# Problem constants (hardcoded; kernel.py must be self-contained)
B, C, H, W = 32, 64, 128, 128
N_CORES = 8
B_LOC = B // N_CORES            # 4 images per core
PACKS = B_LOC // 2              # 2 two-image packs
S = H * W                       # 16384
FG = 160                        # front zero-guard (>=129 needed)
BG = 288
BUF_W = FG + S + BG             # 16832 (16-aligned for DoubleRow AP steps)
Y4 = 4                          # output rows per psum tile
NT = H // Y4                    # 32 row tiles
GRP = 4                         # conv1 tiles per weight-reuse group
NCH = 16                        # load row-chunks (8 rows each)
NHCH = 16                       # repl_h width chunks (fixed DMA issue cost)
CROWS = H // NCH                # 8
NBLK = 4                        # 32-row blocks (edges, repl_a, stores)
BROWS = H // NBLK               # 32
XI = W - 2                      # 126 interior columns
TAPORD = (4, 0, 1, 2, 3, 5, 6, 7, 8)   # Arep group -> tap index (center first)


def _pos(y, x):
    return FG + y * W + x


def _host_weights(w1, w2):
    """Precompute SBUF weight layouts (block-diagonal over image packing)."""
    w1 = np.asarray(w1, np.float32)     # [16, 64, 3, 3]
    w2 = np.asarray(w2, np.float32)     # [1, 16, 3, 3]
    bf = ml_dtypes.bfloat16
    f8 = mybir.dt.np(FP8)

    # conv1 DoubleRow M=128: psum m = dx*64 + seg*32 + il*16 + och, dx in
    # {0,1} (w1A, full-width passes) and the dx=2 taps (w1B, M=64) written
    # into the dx=0 partition block at psum cols [0,510) reading rhs +2:
    # psum0[p] = partial_dx0[p] + partial_dx2[p+2], so
    # h[p] = relu(psum0[p-1] + psum1[p]) needs only a 2-term eviction.
    w1A = np.zeros((128, 2, 3, 128), np.float32)
    w1B = np.zeros((128, 2, 3, 64), np.float32)
    for il in range(2):
        for seg in range(2):
            for ky in range(3):
                for dx in range(2):
                    m0 = dx * 64 + seg * 32 + il * 16
                    w1A[il * 64:(il + 1) * 64, seg, ky, m0:m0 + 16] = \
                        w1[:, :, ky, dx].T
                m0 = seg * 32 + il * 16
                w1B[il * 64:(il + 1) * 64, seg, ky, m0:m0 + 16] = \
                    w1[:, :, ky, 2].T
    # conv1 edges (bf16, per pack): lhsT[p=(i2,c64), t, m=(il2,co16)]
    w1L = np.zeros((128, 9, 32), np.float32)
    for i in range(2):
        for t in range(9):
            ky, kx = divmod(t, 3)
            w1L[i * 64:(i + 1) * 64, t, i * 16:(i + 1) * 16] = w1[:, :, ky, kx].T
    # conv2 DoubleRow: lhsT[p=(g3,il2,c16), seg2, kx, m] = w2[0,c,g,kx] at
    # m = 2*seg + il (M=4: psum rows = (seg,il) = A2 rows 0..3)
    w2D = np.zeros((96, 2, 3, 32), np.float32)
    for g in range(3):
        for il in range(2):
            for seg in range(2):
                for kx in range(3):
                    w2D[g * 32 + il * 16:g * 32 + il * 16 + 16, seg, kx,
                        2 * seg + il] = w2[0, :, g, kx]
    # conv2 edges (fp8 non-DR, per seg): lhsT[p=(g3,i2,c16), kx, il2]
    w2L = np.zeros((96, 3, 2), np.float32)
    for g in range(3):
        for i in range(2):
            for kx in range(3):
                w2L[g * 32 + i * 16:g * 32 + (i + 1) * 16, kx, i] = w2[0, :, g, kx]
    # box+bcast: K=36 over A2 rows (j9, seg2, il2); per-pack lhsT selects seg:
    # onesL[p=(j,seg,il), pk, e, m=(il'2,c64)] =
    #   tap-allowed(t(j), e) * (il' == il) * (seg == pk)
    onesL = np.zeros((36, 2, 3, 128), np.float32)
    for j, t in enumerate(TAPORD):
        kx = t % 3
        for seg in range(2):
            for il in range(2):
                p = 4 * j + 2 * seg + il
                onesL[p, seg, 0, il * 64:(il + 1) * 64] = 1.0
                if kx >= 1:
                    onesL[p, seg, 1, il * 64:(il + 1) * 64] = 1.0
                if kx <= 1:
                    onesL[p, seg, 2, il * 64:(il + 1) * 64] = 1.0
    return (w1A.astype(f8), w1B.astype(f8), w1L.astype(bf), w2D.astype(f8),
            w2L.astype(f8), onesL.astype(bf))


def _view(t, off, rows, cols):
    """AP over [partitions, (rows @ W-stride, cols @ 1)] at free offset `off`."""
    return t[:, off:off + (rows - 1) * W + 128].rearrange(
        "p (y x) -> p y x", x=W)[:, :, :cols]


def _view3(t, seg, off, rows, cols):
    """Same for a [P, 2, BUF_W] tensor at segment `seg`."""
    return t[:, seg, off:off + (rows - 1) * W + 128].rearrange(
        "p (y x) -> p y x", x=W)[:, :, :cols]


def _split(a, b, n=NCH):
    step = (b - a) // n
    cuts = [a + i * step for i in range(n)] + [b]
    return [(cuts[i], cuts[i + 1]) for i in range(n)]


def _build_nc():
    nc = bacc.Bacc(None, target_bir_lowering=False, debug=False)

    x_ext = nc.declare_dram_parameter("x", [B_LOC, C, H, W], F32, isOutput=False)
    out_ext = nc.declare_dram_parameter("out", [B_LOC, C, H, W], F32, isOutput=True)
    w1A_ext = nc.declare_dram_parameter("w1A", [128, 2, 3, 128], FP8, isOutput=False)
    w1B_ext = nc.declare_dram_parameter("w1B", [128, 2, 3, 64], FP8, isOutput=False)
    w1L_ext = nc.declare_dram_parameter("w1L", [128, 9, 32], BF16, isOutput=False)
    w2D_ext = nc.declare_dram_parameter("w2D", [96, 2, 3, 32], FP8, isOutput=False)
    w2L_ext = nc.declare_dram_parameter("w2L", [96, 3, 2], FP8, isOutput=False)
    onesL_ext = nc.declare_dram_parameter("onesL", [36, 2, 3, 128], BF16, isOutput=False)

    with TileContext(nc) as tc:
        with (
            tc.tile_pool(name="wpool", bufs=1) as wpool,
            tc.tile_pool(name="xpool", bufs=2) as xpool,
            tc.tile_pool(name="hpool", bufs=1) as hpool,
            tc.tile_pool(name="bigpool", bufs=2) as bigpool,
            tc.tile_pool(name="aepool", bufs=2) as aepool,
            tc.tile_pool(name="scpool", bufs=4) as scpool,
            tc.tile_pool(name="pspool", bufs=8, space="PSUM") as pspool,
        ):
            w1A = wpool.tile([128, 2, 3, 128], FP8)
            w1B = wpool.tile([128, 2, 3, 64], FP8)
            w1L = wpool.tile([128, 9, 32], BF16)
            w2D = wpool.tile([96, 2, 3, 32], FP8)
            w2L = wpool.tile([96, 3, 2], FP8)
            onesL = wpool.tile([36, 2, 3, 128], BF16)
            for dst, src in ((w1A, w1A_ext), (w1B, w1B_ext), (w1L, w1L_ext),
                             (w2D, w2D_ext), (w2L, w2L_ext), (onesL, onesL_ext)):
                nc.sync.dma_start(out=dst[:], in_=src[:])

            x_sb = [None] * PACKS

            x8 = bigpool.tile([128, 2, BUF_W], FP8, tag="big", name="x8")
            # attention maps: row 4j+2seg+il = tap group j (j=0 center),
            # pack seg, image il. Rows 0..3 written by sigmoid evictions;
            # rows 4..35 are shifted replicas (repl_a). Only rows 0..3 guards
            # are ever read through shifts -> tiny memsets.
            A2 = bigpool.tile([36, BUF_W], BF16, tag="big", name="A2")
            h64 = hpool.tile([64, BUF_W], FP8, name="h64")
            hrep4 = hpool.tile([96, 2, BUF_W], FP8, name="hrep4")

            def guard_memsets():
                nc.vector.memset(x8[:, :, 0:FG], 0.0)
                nc.vector.memset(x8[:, :, FG + S:BUF_W], 0.0)
                nc.vector.memset(A2[0:4, 0:FG], 0.0)
                nc.vector.memset(A2[0:4, FG + S:BUF_W], 0.0)
                for t in (h64, hrep4):
                    nc.vector.memset(t[:, ..., 0:FG], 0.0)
                    nc.vector.memset(t[:, ..., FG + S:BUF_W], 0.0)

            def alloc(p):
                x_sb[p] = xpool.tile([128, BUF_W], BF16, tag="xsb", name=f"xsb{p}")

            def load(p, c):
                # f32 -> bf16 cast DMA (SWDGE); chunks overlap by one row so
                # consumers of rows [8c, 8c+8] depend only on chunks <= c
                r1 = min((c + 1) * CROWS + 1, H)
                nc.gpsimd.dma_start(
                    out=x_sb[p][:, _pos(c * CROWS, 0):_pos(r1, 0)],
                    in_=x_ext[2 * p:2 * p + 2, :, c * CROWS:r1]
                    .rearrange("b c h w -> (b c) (h w)"),
                )

            def cast8(p, c):
                # bf16 -> fp8 copy-cast, split across ScalarE/VectorE
                r1 = min((c + 1) * CROWS + 1, H)
                a, b = _pos(c * CROWS, 0), _pos(r1, 0)
                if (2 * p + c) % 2 == 0:
                    nc.scalar.activation(x8[:, p, a:b], x_sb[p][:, a:b],
                                         mybir.ActivationFunctionType.Copy)
                else:
                    nc.vector.tensor_copy(x8[:, p, a:b], x_sb[p][:, a:b])

            def conv1_dr(rt):
                # 6 passes into ONE psum bank: 3 ky passes at M=128 (dx 0/1)
                # full width, then 3 ky passes at M=64 (dx=2) into the dx=0
                # partition block at cols [0,510) reading the rhs at +2.
                pa = pspool.tile([128, 512], F32, tag="ps", name="pa")
                for ky in range(3):
                    q = _pos(rt * Y4 + ky - 1, 0)
                    nc.tensor.matmul(
                        pa[:, :],
                        w1A[:, :, ky, :],
                        x8[:, :, q:q + 512],
                        perf_mode=mybir.MatmulPerfMode.DoubleRow,
                        start=(ky == 0), stop=False,
                    )
                for ky in range(3):
                    q = _pos(rt * Y4 + ky - 1, 2)
                    nc.tensor.matmul(
                        pa[0:64, 0:510],
                        w1B[:, :, ky, :],
                        x8[:, :, q:q + 510],
                        perf_mode=mybir.MatmulPerfMode.DoubleRow,
                        start=False, stop=(ky == 2),
                    )
                # h[p] = relu(pa0[p-1] + pa1[p]) for flat p in [1,511);
                # cols x=0/127 are wrap-garbage, overwritten by the edge path.
                # DVE can read only ONE psum input -> 3-op pipeline across
                # ACT (psum copy), DVE (add), Pool (relu + fp8 cast).
                q0 = _pos(rt * Y4, 0)
                t1 = scpool.tile([64, 510], BF16, tag="sc", name="t1")
                nc.scalar.activation(t1[:, :], pa[64:128, 1:511],
                                     mybir.ActivationFunctionType.Copy)
                nc.vector.tensor_add(t1[:, :], pa[0:64, 0:510], t1[:, :])
                nc.vector.tensor_scalar_max(
                    h64[:, q0 + 1:q0 + 511], t1[:, :], 0.0)

            def conv1_edges(p, u):
                # both edge columns for 16 rows share one psum bank
                xs = x_sb[p]
                yc = u * 16
                pe = pspool.tile([32, 2, 16], F32, tag="ps", name="pe")
                for half, (col, kxs) in enumerate(((0, (1, 2)), (W - 1, (0, 1)))):
                    first = True
                    for ky in range(3):
                        for kx in kxs:
                            nc.tensor.matmul(
                                pe[:, half, :].rearrange("p (y x) -> p y x", x=1),
                                w1L[:, ky * 3 + kx, :],
                                _view(xs, _pos(yc + ky - 1, col + kx - 1), 16, 1),
                                start=first, stop=(ky == 2 and kx == kxs[-1]),
                            )
                            first = False
                # ONE relu+cast for both edge columns
                dst = h64[32 * p:32 * p + 32,
                          _pos(yc, 0):_pos(yc + 15, W - 1) + 1].rearrange(
                    "p (y x) -> p y x", x=W)[:, :, 0:W:W - 1]
                nc.vector.tensor_scalar_max(
                    dst, pe[:, :, :].rearrange("p h y -> p y h"), 0.0)

            def repl_h(c):
                # Hrep4[(g,il,c), s][pos] = h_seg_s[pos + (g-1)*W]
                # 4 coarse chunks: HWDGE issue cost is fixed per DMA
                for s in range(2):
                    a0, b0 = _split(W, BUF_W, NHCH)[c]
                    nc.sync.dma_start(out=hrep4[0:32, s, a0:b0],
                                      in_=h64[32 * s:32 * s + 32, a0 - W:b0 - W])
                    a1, b1 = _split(0, BUF_W, NHCH)[c]
                    nc.sync.dma_start(out=hrep4[32:64, s, a1:b1],
                                      in_=h64[32 * s:32 * s + 32, a1:b1])
                    a2, b2 = _split(0, BUF_W - W, NHCH)[c]
                    nc.sync.dma_start(out=hrep4[64:96, s, a2:b2],
                                      in_=h64[32 * s:32 * s + 32, a2 + W:b2 + W])

            def conv2(g2):
                # dx-outer over pairs of tiles: weight loads amortize
                tiles = [2 * g2, 2 * g2 + 1]
                pzs = [pspool.tile([32, 512], F32, tag="ps", name="pz")
                       for _ in tiles]
                for kx in range(3):
                    for j, rt in enumerate(tiles):
                        q = _pos(rt * Y4, kx - 1)
                        nc.tensor.matmul(
                            pzs[j][:, :],
                            w2D[:, :, kx, :],
                            hrep4[:, :, q:q + 512],
                            perf_mode=mybir.MatmulPerfMode.DoubleRow,
                            start=(kx == 0), stop=(kx == 2),
                        )
                for j, rt in enumerate(tiles):
                    y0 = rt * Y4
                    # ONE sigmoid for all 4 images (edge cols garbage; fixed
                    # below)
                    nc.scalar.activation(
                        A2[0:4, _pos(y0, 0):_pos(y0, 0) + 512],
                        pzs[j][0:4, :],
                        mybir.ActivationFunctionType.Sigmoid,
                    )

            ae_cur = [None]

            def conv2_edges(u):
                # all 4 (pack, half) edge strips share one psum tile at
                # partition base 0: [2=(il), pk2, half2, y16]
                yc = u * 16
                pz = pspool.tile([2, 2, 2, 16], F32, tag="ps", name="pze")
                for p in range(PACKS):
                    for half, (col, kxs) in enumerate(((0, (1, 2)),
                                                       (W - 1, (0, 1)))):
                        for j, kx in enumerate(kxs):
                            nc.tensor.matmul(
                                pz[:, p, half, :]
                                .rearrange("p (y x) -> p y x", x=1),
                                w2L[:, kx, :],
                                _view3(hrep4, p, _pos(yc, col + kx - 1), 16, 1),
                                start=(j == 0), stop=(j == len(kxs) - 1),
                            )
                # ONE sigmoid for all 4 strips into scratch, then 2 DMAs
                # scatter into A2 rows (which need base-2 access -> DMA only)
                ae = aepool.tile([2, 2, 2, 16], BF16, name="ae")
                nc.scalar.activation(
                    ae[:, :, :, :], pz[:, :, :, :],
                    mybir.ActivationFunctionType.Sigmoid,
                )
                for p in range(PACKS):
                    for half, col in enumerate((0, W - 1)):
                        dst = _view(A2, _pos(yc, col), 16, 1)[2 * p:2 * p + 2]
                        nc.scalar.dma_start(
                            out=dst,
                            in_=ae[:, p, half, :].rearrange(
                                "i (y x) -> i y x", x=1))

            def repl_a(b):
                # tap j for all 4 images at once: [4, L] shifted self-copies
                for j in range(1, 9):
                    t = TAPORD[j]
                    o = (t // 3 - 1) * W + (t % 3 - 1)
                    if o > 0:
                        a, e = _split(0, BUF_W - o, NBLK)[b]
                    else:
                        a, e = _split(-o, BUF_W, NBLK)[b]
                    nc.gpsimd.dma_start(out=A2[4 * j:4 * j + 4, a:e],
                                        in_=A2[0:4, a + o:e + o])

            def box_mul(p, rt, reuse_w=False):
                xs = x_sb[p]
                y0 = rt * Y4
                pb = pspool.tile([128, 504], F32, tag="ps", name="pb")
                bi = nc.tensor.matmul(
                    pb[:, :].rearrange("p (y x) -> p y x", y=Y4),
                    onesL[:, p, 0, :],
                    _view(A2, _pos(y0, 1), Y4, XI)[0:36],
                    start=True, stop=True,
                )
                if reuse_w:
                    bi.ins.ldweights = False
                v = _view(xs, _pos(y0, 1), Y4, XI)
                nc.vector.tensor_mul(
                    v, v, pb[:, :].rearrange("p (y x) -> p y x", y=Y4))

            def box_mul_edges(p, u):
                xs = x_sb[p]
                yc = u * 16
                pb = pspool.tile([128, 2, 16], F32, tag="ps", name="pbe")
                for half, (e, col) in enumerate(((1, 0), (2, W - 1))):
                    nc.tensor.matmul(
                        pb[:, half, :].rearrange("p (y x) -> p y x", x=1),
                        onesL[:, p, e, :],
                        _view(A2, _pos(yc, col), 16, 1)[0:36],
                        start=True, stop=True,
                    )
                # ONE multiply for both edge columns
                v = xs[:, _pos(yc, 0):_pos(yc + 15, W - 1) + 1].rearrange(
                    "p (y x) -> p y x", x=W)[:, :, 0:W:W - 1]
                nc.vector.tensor_mul(
                    v, v, pb[:, :, :].rearrange("p h y -> p y h"))

            def store(p, b):
                # bf16 -> f32 cast DMA (SWDGE), rows [b*32, (b+1)*32)
                nc.gpsimd.dma_start(
                    out=out_ext[2 * p:2 * p + 2, :, b * BROWS:(b + 1) * BROWS]
                    .rearrange("b c h w -> (b c) (h w)"),
                    in_=x_sb[p][:, _pos(b * BROWS, 0):_pos((b + 1) * BROWS, 0)],
                )

            # ---- emission: software pipeline over 16-row ticks ----
            # tick u: conv1(u) | repl_h(u-1) | conv2(u-2) | repl_a(u-4, 32-row)
            #         | box+mul(u-6) | store(u-7, 32-row)
            alloc(0)
            alloc(1)
            for p in range(PACKS):
                nc.vector.memset(x_sb[p][:, 0:FG], 0.0)
                nc.vector.memset(x_sb[p][:, FG + S:BUF_W], 0.0)
            guard_memsets()
            for c in range(NCH):
                load(0, c)
                load(1, c)
                cast8(0, c)
                cast8(1, c)

            NTK = 8   # 16-row ticks
            for u in range(NTK + 8):
                # emission order IS program order for the scheduler's
                # read-after-write semantics: keep producer chain order
                # conv1 -> repl_h -> conv2 -> repl_a -> box -> store.
                if u < NTK:
                    for rt in range(4 * u, 4 * u + 4):
                        conv1_dr(rt)
                    conv1_edges(0, u)
                    conv1_edges(1, u)
                v = u - 1
                if 0 <= v < NTK:
                    repl_h(2 * v)
                    repl_h(2 * v + 1)
                v = u - 2
                if 0 <= v < NTK:
                    conv2(2 * v)
                    conv2(2 * v + 1)
                    conv2_edges(v)
                v = u - 4
                if 0 <= v < NTK and v % 2 == 0:
                    repl_a(v // 2)
                v = u - 6
                if 0 <= v < NTK:
                    for p in range(PACKS):
                        for i, rt in enumerate(range(4 * v, 4 * v + 4)):
                            box_mul(p, rt, reuse_w=(i > 0))
                        box_mul_edges(p, v)
                v = u - 7
                if 0 <= v < NTK and v % 2 == 1:
                    store(0, v // 2)
                    store(1, v // 2)

    nc.compile()
    return nc


_CACHE = {}


def _get_nc():
    if "nc" not in _CACHE:
        _CACHE["nc"] = _build_nc()
    return _CACHE["nc"]


def _reset_device():
    """Best-effort axon terminal reset (recovers NRT_EXEC_UNIT_UNRECOVERABLE)."""
    try:
        import ctypes

        lib = ctypes.CDLL("/opt/axon/libaxon_pjrt.so")
        lib.axon_reset.restype = ctypes.c_int64
        lib.axon_reset()
    except Exception:
        pass


def _run(x, w1, w2, trace=False):
    x = np.ascontiguousarray(np.asarray(x, np.float32))
    w1A, w1B, w1L, w2D, w2L, onesL = _host_weights(w1, w2)
    nc = _get_nc()
    in_maps = []
    for k in range(N_CORES):
        in_maps.append({
            "x": x[k * B_LOC:(k + 1) * B_LOC],
            "w1A": w1A, "w1B": w1B, "w1L": w1L, "w2D": w2D, "w2L": w2L,
            "onesL": onesL,
        })
    try:
        res = run_bass_kernel_spmd(nc, in_maps, core_ids=list(range(N_CORES)),
                                   trace=trace)
    except Exception as e:
        if "unrecoverable" not in str(e).lower():
            raise
        _reset_device()
        res = run_bass_kernel_spmd(nc, in_maps, core_ids=list(range(N_CORES)),
                                   trace=trace)
    out = np.concatenate([r["out"] for r in res.results], axis=0)
    return out.astype(np.float32), res


def kernel(x, weights, w1, w2):
    out, _ = _run(x, w1, w2, trace=False)
    return out


def kernel_timed(x, weights, w1, w2):
    out, res = _run(x, w1, w2, trace=True)
    return out, res.exec_time_ns

